# revision 33
# baseline (speedup 1.0000x reference)
"""Trainium2 Bass kernel for nn_MultLayerAdaptiveSimple.

Computes out = X * W[idx, 0] + Y * W[idx, 1] where idx = reward[..., 0]
(values in {0, 1}), X/Y: [4, 4096, 2048] f32, W: [2, 2] f32.

Sharding: pure data-parallel over the flattened (B*S) row axis across 8
NeuronCores; the 2x2 table is replicated. Each core processes 2048 rows
of 2048 f32 elements (16 MB per tensor per core).

Device work per core:
  - per-row blend weights a = W[idx,0], b = W[idx,1] computed exactly on
    DVE via a = (1-idx)*W00 + idx*W10 (idx in {0,1} so each product is
    exact), using per-partition scalar operands.
  - per 128-row chunk: ACT does y *= b (activation Copy with per-partition
    scale), DVE does x = (x * a) + y in one fused scalar_tensor_tensor.
  - HWDGE (nc.sync) moves 4 MB tiles HBM<->SBUF.
"""

import numpy as np

import concourse.bacc as bacc
import concourse.bass as bass
import concourse.mybir as mybir
from concourse.bass_utils import run_bass_kernel_spmd
from concourse.tile import TileContext

B, S, D = 4, 4096, 2048
N_CORES = 8
ROWS = B * S                      # 16384
ROWS_PER_CORE = ROWS // N_CORES   # 2048
P = 128                           # SBUF partitions
GROUPS = ROWS_PER_CORE // P       # 16 row-groups of 128 rows per core
# DMA tile plan: (first_group, n_groups) per tile. Uniform 2 MB tiles
# measured fastest: 1 MB items slow the ring drain (per-item overhead),
# 4 MB mid-tiles measured no better and delay the first compute.
TILE_PLAN = [(g, 2) for g in range(0, GROUPS, 2)]

F32 = mybir.dt.float32
MULT = mybir.AluOpType.mult
ADD = mybir.AluOpType.add


def _build_bass() -> bass.Bass:
    nc = bacc.Bacc(trn_type="TRN2", debug=False, enable_partition_id=False)

    x = nc.dram_tensor("x", [ROWS_PER_CORE, D], F32, kind="ExternalInput").ap()
    y = nc.dram_tensor("y", [ROWS_PER_CORE, D], F32, kind="ExternalInput").ap()
    idx = nc.dram_tensor("idx", [P, GROUPS], F32, kind="ExternalInput").ap()
    w = nc.dram_tensor("w", [P, 4], F32, kind="ExternalInput").ap()
    out = nc.dram_tensor("out", [ROWS_PER_CORE, D], F32, kind="ExternalOutput").ap()

    # Group g covers rows [g*P, (g+1)*P): partition p holds row g*P + p,
    # matching idx[:, g].
    xg = x.rearrange("(g p) d -> g p d", p=P)
    yg = y.rearrange("(g p) d -> g p d", p=P)
    ov = out.rearrange("(g p) d -> g p d", p=P)

    def tile_view(src_g, g0, ch):
        # AP [P, ch, D] covering row-groups g0..g0+ch-1
        v = src_g[g0 : g0 + ch]  # [ch, P, D]
        return v.rearrange("c p d -> p c d")

    with TileContext(nc) as tc:
        with (
            tc.tile_pool(name="small", bufs=1) as small,
            tc.tile_pool(name="xp", bufs=5) as xp,
            tc.tile_pool(name="yp", bufs=5) as yp,
        ):
            idx_t = small.tile([P, GROUPS], F32)
            w_t = small.tile([P, 4], F32)
            # On the SWDGE queue (idle until stores begin): tiny strided
            # transfers at the head of a HWDGE load ring would FIFO-delay
            # the first 2MB data loads by ~10us.
            nc.gpsimd.dma_start(out=idx_t[:], in_=idx)
            nc.gpsimd.dma_start(out=w_t[:], in_=w)

            # nidx = 1 - idx (exact for idx in {0,1})
            nidx_t = small.tile([P, GROUPS], F32)
            nc.vector.tensor_scalar(nidx_t[:], idx_t[:], -1.0, 1.0, MULT, ADD)

            # a = nidx*W00 + idx*W10 ; b = nidx*W01 + idx*W11   (all exact)
            ta = small.tile([P, GROUPS], F32)
            tb = small.tile([P, GROUPS], F32)
            a_t = small.tile([P, GROUPS], F32)
            b_t = small.tile([P, GROUPS], F32)
            nc.vector.tensor_scalar(ta[:], idx_t[:], w_t[:, 2:3], None, MULT)
            nc.vector.scalar_tensor_tensor(a_t[:], nidx_t[:], w_t[:, 0:1], ta[:], MULT, ADD)
            nc.vector.tensor_scalar(tb[:], idx_t[:], w_t[:, 3:4], None, MULT)
            nc.vector.scalar_tensor_tensor(b_t[:], nidx_t[:], w_t[:, 1:2], tb[:], MULT, ADD)

            for g0, ch in TILE_PLAN:
                xt = xp.tile([P, 2 * D], F32, tag="xt")
                yt = yp.tile([P, 2 * D], F32, tag="yt")
                # x loads on the SP HWDGE ring, y loads on the ACT HWDGE
                # ring, stores on the SWDGE (gpsimd) queue: three DMA
                # streams that overlap instead of serializing in one FIFO.
                # Each issuing engine is a pure dispatcher: a stalled
                # compute op in a dispatcher's stream would head-of-line-
                # block its queue, so all compute lives on DVE.
                dst = xt[:, : ch * D].rearrange("p (c d) -> p c d", c=ch)
                nc.sync.dma_start(out=dst, in_=tile_view(xg, g0, ch))
                dst = yt[:, : ch * D].rearrange("p (c d) -> p c d", c=ch)
                nc.scalar.dma_start(out=dst, in_=tile_view(yg, g0, ch))
                for c in range(ch):
                    g = g0 + c
                    xs = xt[:, c * D : (c + 1) * D]
                    ys = yt[:, c * D : (c + 1) * D]
                    nc.vector.tensor_scalar(ys, ys, b_t[:, g : g + 1], None, MULT)
                    nc.vector.scalar_tensor_tensor(
                        xs, xs, a_t[:, g : g + 1], ys, MULT, ADD
                    )
                    # store immediately; the final two stores go on the
                    # HWDGE rings, idle once the last loads have drained.
                    if g == GROUPS - 1:
                        nc.sync.dma_start(out=ov[g], in_=xs)
                    elif g == GROUPS - 2:
                        nc.scalar.dma_start(out=ov[g], in_=xs)
                    else:
                        nc.gpsimd.dma_start(out=ov[g], in_=xs)

    nc.compile()
    return nc


def _shard_inputs(X, Y, reward, W):
    Xf = np.ascontiguousarray(np.asarray(X, dtype=np.float32).reshape(ROWS, D))
    Yf = np.ascontiguousarray(np.asarray(Y, dtype=np.float32).reshape(ROWS, D))
    idx_all = np.asarray(reward).reshape(ROWS).astype(np.float32)
    w_rep = np.ascontiguousarray(
        np.tile(np.asarray(W, dtype=np.float32).reshape(1, 4), (P, 1))
    )
    in_maps = []
    for k in range(N_CORES):
        sl = slice(k * ROWS_PER_CORE, (k + 1) * ROWS_PER_CORE)
        # idx_core[p, g] = idx of row g*P + p of this core's shard
        idx_core = np.ascontiguousarray(idx_all[sl].reshape(GROUPS, P).T)
        in_maps.append(
            {
                "x": np.ascontiguousarray(Xf[sl]),
                "y": np.ascontiguousarray(Yf[sl]),
                "idx": idx_core,
                "w": w_rep,
            }
        )
    return in_maps


def run(X, Y, reward, W, trace=False, tmpdir=None):
    """Build, run on 8 cores; returns (full_output, BassKernelResults)."""
    in_maps = _shard_inputs(X, Y, reward, W)
    nc = _build_bass()
    res = run_bass_kernel_spmd(
        nc, in_maps, core_ids=list(range(N_CORES)), trace=trace, tmpdir=tmpdir
    )
    shards = [res.results[k]["out"] for k in range(N_CORES)]
    full = np.concatenate(shards, axis=0).reshape(B, S, D)
    return full, res


def kernel(X, Y, reward, W):
    full, _ = run(X, Y, reward, W)
    return full


# revision 35
# speedup vs baseline: 1.1328x; 1.1328x over previous
"""Trainium2 Bass kernel for nn_MultLayerAdaptiveSimple.

Computes out = X * W[idx, 0] + Y * W[idx, 1] where idx = reward[..., 0]
(values in {0, 1}), X/Y: [4, 4096, 2048] f32, W: [2, 2] f32.

Sharding: pure data-parallel over the flattened (B*S) row axis across 8
NeuronCores; the 2x2 table is replicated. Each core processes 2048 rows
of 2048 f32 elements (16 MB per tensor per core).

Device work per core:
  - per-row blend weights a = W[idx,0], b = W[idx,1] computed exactly on
    DVE via a = (1-idx)*W00 + idx*W10 (idx in {0,1} so each product is
    exact), using per-partition scalar operands.
  - per 128-row chunk: ACT does y *= b (activation Copy with per-partition
    scale), DVE does x = (x * a) + y in one fused scalar_tensor_tensor.
  - HWDGE (nc.sync) moves 4 MB tiles HBM<->SBUF.
"""

import numpy as np

import concourse.bacc as bacc
import concourse.bass as bass
import concourse.mybir as mybir
from concourse.bass_utils import run_bass_kernel_spmd
from concourse.tile import TileContext

B, S, D = 4, 4096, 2048
N_CORES = 8
ROWS = B * S                      # 16384
ROWS_PER_CORE = ROWS // N_CORES   # 2048
P = 128                           # SBUF partitions
GROUPS = ROWS_PER_CORE // P       # 16 row-groups of 128 rows per core
# DMA tile plan: (first_group, n_groups) per tile. Uniform 2 MB tiles
# measured fastest: 1 MB items slow the ring drain (per-item overhead),
# 4 MB mid-tiles measured no better and delay the first compute.
TILE_PLAN = [(g, 2) for g in range(0, GROUPS, 2)]

F32 = mybir.dt.float32
MULT = mybir.AluOpType.mult
ADD = mybir.AluOpType.add


def _build_bass() -> bass.Bass:
    nc = bacc.Bacc(trn_type="TRN2", debug=False, enable_partition_id=False)

    x = nc.dram_tensor("x", [ROWS_PER_CORE, D], F32, kind="ExternalInput").ap()
    y = nc.dram_tensor("y", [ROWS_PER_CORE, D], F32, kind="ExternalInput").ap()
    idx = nc.dram_tensor("idx", [P, GROUPS], F32, kind="ExternalInput").ap()
    w = nc.dram_tensor("w", [P, 4], F32, kind="ExternalInput").ap()
    out = nc.dram_tensor("out", [ROWS_PER_CORE, D], F32, kind="ExternalOutput").ap()

    # Group g covers rows [g*P, (g+1)*P): partition p holds row g*P + p,
    # matching idx[:, g].
    xv2 = x.rearrange("(t c p) d -> t p c d", c=2, p=P)
    yv2 = y.rearrange("(t c p) d -> t p c d", c=2, p=P)
    ov = out.rearrange("(g p) d -> g p d", p=P)

    with TileContext(nc) as tc:
        with (
            tc.tile_pool(name="small", bufs=1) as small,
            tc.tile_pool(name="xp", bufs=5) as xp,
            tc.tile_pool(name="yp", bufs=5) as yp,
        ):
            idx_t = small.tile([P, GROUPS], F32)
            w_t = small.tile([P, 4], F32)
            # On the SWDGE queue (idle until stores begin): tiny strided
            # transfers at the head of a HWDGE load ring would FIFO-delay
            # the first 2MB data loads by ~10us.
            nc.gpsimd.dma_start(out=idx_t[:], in_=idx)
            nc.gpsimd.dma_start(out=w_t[:], in_=w)

            # nidx = 1 - idx (exact for idx in {0,1})
            nidx_t = small.tile([P, GROUPS], F32)
            nc.vector.tensor_scalar(nidx_t[:], idx_t[:], -1.0, 1.0, MULT, ADD)

            # a = nidx*W00 + idx*W10 ; b = nidx*W01 + idx*W11   (all exact)
            ta = small.tile([P, GROUPS], F32)
            tb = small.tile([P, GROUPS], F32)
            a_t = small.tile([P, GROUPS], F32)
            b_t = small.tile([P, GROUPS], F32)
            nc.vector.tensor_scalar(ta[:], idx_t[:], w_t[:, 2:3], None, MULT)
            nc.vector.scalar_tensor_tensor(a_t[:], nidx_t[:], w_t[:, 0:1], ta[:], MULT, ADD)
            nc.vector.tensor_scalar(tb[:], idx_t[:], w_t[:, 3:4], None, MULT)
            nc.vector.scalar_tensor_tensor(b_t[:], nidx_t[:], w_t[:, 1:2], tb[:], MULT, ADD)

            for g0, ch in TILE_PLAN:
                xt = xp.tile([P, 2 * D], F32, tag="xt")
                yt = yp.tile([P, 2 * D], F32, tag="yt")
                # x loads on the SP HWDGE ring, y loads on the ACT HWDGE
                # ring, stores on the SWDGE (gpsimd) queue: three DMA
                # streams that overlap instead of serializing in one FIFO.
                # Each issuing engine is a pure dispatcher: a stalled
                # compute op in a dispatcher's stream would head-of-line-
                # block its queue, so all compute lives on DVE.
                nc.sync.dma_start(
                    out=xt[:].rearrange("p (c d) -> p c d", c=2), in_=xv2[g0 // 2]
                )
                nc.scalar.dma_start(
                    out=yt[:].rearrange("p (c d) -> p c d", c=2), in_=yv2[g0 // 2]
                )
                for c in range(ch):
                    g = g0 + c
                    xs = xt[:, c * D : (c + 1) * D]
                    ys = yt[:, c * D : (c + 1) * D]
                    nc.vector.tensor_scalar(ys, ys, b_t[:, g : g + 1], None, MULT)
                    nc.vector.scalar_tensor_tensor(
                        xs, xs, a_t[:, g : g + 1], ys, MULT, ADD
                    )
                    # store immediately; the final two stores go on the
                    # HWDGE rings, idle once the last loads have drained.
                    if g == GROUPS - 1:
                        nc.sync.dma_start(out=ov[g], in_=xs)
                    elif g == GROUPS - 2:
                        nc.scalar.dma_start(out=ov[g], in_=xs)
                    else:
                        nc.gpsimd.dma_start(out=ov[g], in_=xs)

    nc.compile()
    return nc


def _shard_inputs(X, Y, reward, W):
    Xf = np.ascontiguousarray(np.asarray(X, dtype=np.float32).reshape(ROWS, D))
    Yf = np.ascontiguousarray(np.asarray(Y, dtype=np.float32).reshape(ROWS, D))
    idx_all = np.asarray(reward).reshape(ROWS).astype(np.float32)
    w_rep = np.ascontiguousarray(
        np.tile(np.asarray(W, dtype=np.float32).reshape(1, 4), (P, 1))
    )
    in_maps = []
    for k in range(N_CORES):
        sl = slice(k * ROWS_PER_CORE, (k + 1) * ROWS_PER_CORE)
        # idx_core[p, g] = idx of row g*P + p of this core's shard
        idx_core = np.ascontiguousarray(idx_all[sl].reshape(GROUPS, P).T)
        in_maps.append(
            {
                "x": np.ascontiguousarray(Xf[sl]),
                "y": np.ascontiguousarray(Yf[sl]),
                "idx": idx_core,
                "w": w_rep,
            }
        )
    return in_maps


def run(X, Y, reward, W, trace=False, tmpdir=None):
    """Build, run on 8 cores; returns (full_output, BassKernelResults)."""
    in_maps = _shard_inputs(X, Y, reward, W)
    nc = _build_bass()
    res = run_bass_kernel_spmd(
        nc, in_maps, core_ids=list(range(N_CORES)), trace=trace, tmpdir=tmpdir
    )
    shards = [res.results[k]["out"] for k in range(N_CORES)]
    full = np.concatenate(shards, axis=0).reshape(B, S, D)
    return full, res


def kernel(X, Y, reward, W):
    full, _ = run(X, Y, reward, W)
    return full


# revision 40
# speedup vs baseline: 1.1384x; 1.0049x over previous
"""Trainium2 Bass kernel for nn_MultLayerAdaptiveSimple.

Computes out = X * W[idx, 0] + Y * W[idx, 1] where idx = reward[..., 0]
(values in {0, 1}), X/Y: [4, 4096, 2048] f32, W: [2, 2] f32.

Sharding: pure data-parallel over the flattened (B*S) row axis across 8
NeuronCores; the 2x2 table is replicated. Each core processes 2048 rows
of 2048 f32 elements (16 MB per tensor per core).

Device work per core:
  - per-row blend weights a = W[idx,0], b = W[idx,1] computed exactly on
    DVE via a = (1-idx)*W00 + idx*W10 (idx in {0,1} so each product is
    exact), using per-partition scalar operands.
  - per 128-row chunk (all on DVE): y *= b (tensor_scalar), then
    x = (x * a) + y in one fused scalar_tensor_tensor; stored in place.
  - three concurrent DMA streams: x loads on the SP HWDGE ring (nc.sync),
    y loads on the ACT HWDGE ring (nc.scalar), stores on the SWDGE queue
    (nc.gpsimd); 2 MB load tiles, 1 MB chunk stores, the final two stores
    on the by-then-idle HWDGE rings. Each issuing engine is a pure
    dispatcher (no data-dependent compute) to avoid head-of-line blocking.

Measured (8 cores, NTFF profile): 132.2-132.7 us HW exec, bit-exact vs
the reference (abs err 0.0); ~380 GB/s/core end-to-end, ~425 GB/s
mid-kernel aggregate DMA.
"""

import numpy as np

import concourse.bacc as bacc
import concourse.bass as bass
import concourse.mybir as mybir
from concourse.bass_utils import run_bass_kernel_spmd
from concourse.tile import TileContext

B, S, D = 4, 4096, 2048
N_CORES = 8
ROWS = B * S                      # 16384
ROWS_PER_CORE = ROWS // N_CORES   # 2048
P = 128                           # SBUF partitions
GROUPS = ROWS_PER_CORE // P       # 16 row-groups of 128 rows per core
# DMA tile plan: (first_group, n_groups) per tile. 2 MB tiles for ring
# throughput (1 MB items everywhere slow the ring drain; 4 MB mid-tiles
# measured no better), except the LAST tile is split into two 1 MB items
# so the final compute chunk is gated on a 1 MB arrival, shortening the
# post-load serial tail (last-load -> last-STT -> last-store).
TILE_PLAN = [(g, 2) for g in range(0, GROUPS - 2, 2)] + [(14, 1), (15, 1)]

F32 = mybir.dt.float32
MULT = mybir.AluOpType.mult
ADD = mybir.AluOpType.add


def _build_bass() -> bass.Bass:
    nc = bacc.Bacc(trn_type="TRN2", debug=False, enable_partition_id=False)

    x = nc.dram_tensor("x", [ROWS_PER_CORE, D], F32, kind="ExternalInput").ap()
    y = nc.dram_tensor("y", [ROWS_PER_CORE, D], F32, kind="ExternalInput").ap()
    idx = nc.dram_tensor("idx", [P, GROUPS], F32, kind="ExternalInput").ap()
    w = nc.dram_tensor("w", [P, 4], F32, kind="ExternalInput").ap()
    out = nc.dram_tensor("out", [ROWS_PER_CORE, D], F32, kind="ExternalOutput").ap()

    # Group g covers rows [g*P, (g+1)*P): partition p holds row g*P + p,
    # matching idx[:, g].
    xv2 = x.rearrange("(t c p) d -> t p c d", c=2, p=P)
    yv2 = y.rearrange("(t c p) d -> t p c d", c=2, p=P)
    xv1 = x.rearrange("(g p) d -> g p d", p=P)
    yv1 = y.rearrange("(g p) d -> g p d", p=P)
    ov = out.rearrange("(g p) d -> g p d", p=P)

    with TileContext(nc) as tc:
        with (
            tc.tile_pool(name="small", bufs=1) as small,
            tc.tile_pool(name="xp", bufs=5) as xp,
            tc.tile_pool(name="yp", bufs=5) as yp,
        ):
            idx_t = small.tile([P, GROUPS], F32)
            w_t = small.tile([P, 4], F32)
            # On the SWDGE queue (idle until stores begin): tiny strided
            # transfers at the head of a HWDGE load ring would FIFO-delay
            # the first 2MB data loads by ~10us.
            nc.gpsimd.dma_start(out=idx_t[:], in_=idx)
            nc.gpsimd.dma_start(out=w_t[:], in_=w)

            # nidx = 1 - idx (exact for idx in {0,1})
            nidx_t = small.tile([P, GROUPS], F32)
            nc.vector.tensor_scalar(nidx_t[:], idx_t[:], -1.0, 1.0, MULT, ADD)

            # a = nidx*W00 + idx*W10 ; b = nidx*W01 + idx*W11   (all exact)
            ta = small.tile([P, GROUPS], F32)
            tb = small.tile([P, GROUPS], F32)
            a_t = small.tile([P, GROUPS], F32)
            b_t = small.tile([P, GROUPS], F32)
            nc.vector.tensor_scalar(ta[:], idx_t[:], w_t[:, 2:3], None, MULT)
            nc.vector.scalar_tensor_tensor(a_t[:], nidx_t[:], w_t[:, 0:1], ta[:], MULT, ADD)
            nc.vector.tensor_scalar(tb[:], idx_t[:], w_t[:, 3:4], None, MULT)
            nc.vector.scalar_tensor_tensor(b_t[:], nidx_t[:], w_t[:, 1:2], tb[:], MULT, ADD)

            for g0, ch in TILE_PLAN:
                xt = xp.tile([P, 2 * D], F32, tag="xt")
                yt = yp.tile([P, 2 * D], F32, tag="yt")
                # x loads on the SP HWDGE ring, y loads on the ACT HWDGE
                # ring, stores on the SWDGE (gpsimd) queue: three DMA
                # streams that overlap instead of serializing in one FIFO.
                # Each issuing engine is a pure dispatcher: a stalled
                # compute op in a dispatcher's stream would head-of-line-
                # block its queue, so all compute lives on DVE.
                if ch == 2:
                    nc.sync.dma_start(
                        out=xt[:].rearrange("p (c d) -> p c d", c=2), in_=xv2[g0 // 2]
                    )
                    nc.scalar.dma_start(
                        out=yt[:].rearrange("p (c d) -> p c d", c=2), in_=yv2[g0 // 2]
                    )
                else:
                    nc.sync.dma_start(out=xt[:, :D], in_=xv1[g0])
                    nc.scalar.dma_start(out=yt[:, :D], in_=yv1[g0])
                for c in range(ch):
                    g = g0 + c
                    xs = xt[:, c * D : (c + 1) * D]
                    ys = yt[:, c * D : (c + 1) * D]
                    nc.vector.tensor_scalar(ys, ys, b_t[:, g : g + 1], None, MULT)
                    nc.vector.scalar_tensor_tensor(
                        xs, xs, a_t[:, g : g + 1], ys, MULT, ADD
                    )
                    # Store immediately. Only the very last store rides a
                    # HWDGE ring (idle once loads drain): a ring-store
                    # issued before a later load dispatch on the same
                    # engine would head-of-line-block that load's FIFO.
                    if g == GROUPS - 1:
                        nc.sync.dma_start(out=ov[g], in_=xs)
                    else:
                        nc.gpsimd.dma_start(out=ov[g], in_=xs)

    nc.compile()
    return nc


def _shard_inputs(X, Y, reward, W):
    Xf = np.ascontiguousarray(np.asarray(X, dtype=np.float32).reshape(ROWS, D))
    Yf = np.ascontiguousarray(np.asarray(Y, dtype=np.float32).reshape(ROWS, D))
    idx_all = np.asarray(reward).reshape(ROWS).astype(np.float32)
    w_rep = np.ascontiguousarray(
        np.tile(np.asarray(W, dtype=np.float32).reshape(1, 4), (P, 1))
    )
    in_maps = []
    for k in range(N_CORES):
        sl = slice(k * ROWS_PER_CORE, (k + 1) * ROWS_PER_CORE)
        # idx_core[p, g] = idx of row g*P + p of this core's shard
        idx_core = np.ascontiguousarray(idx_all[sl].reshape(GROUPS, P).T)
        in_maps.append(
            {
                "x": np.ascontiguousarray(Xf[sl]),
                "y": np.ascontiguousarray(Yf[sl]),
                "idx": idx_core,
                "w": w_rep,
            }
        )
    return in_maps


def run(X, Y, reward, W, trace=False, tmpdir=None):
    """Build, run on 8 cores; returns (full_output, BassKernelResults)."""
    in_maps = _shard_inputs(X, Y, reward, W)
    nc = _build_bass()
    res = run_bass_kernel_spmd(
        nc, in_maps, core_ids=list(range(N_CORES)), trace=trace, tmpdir=tmpdir
    )
    shards = [res.results[k]["out"] for k in range(N_CORES)]
    full = np.concatenate(shards, axis=0).reshape(B, S, D)
    return full, res


def kernel(X, Y, reward, W):
    full, _ = run(X, Y, reward, W)
    return full
